# revision 2
# baseline (speedup 1.0000x reference)
"""Bass TRN2 kernel for the boundary cosine-similarity context loss.

Decomposition (per core, 8 cores; batch b = k//2, row-half h = k%2):
190 produced rows in 19 blocks x 10 rows; host pre-converts features to
bf16.  Only 12 of the 24 window shifts are computed; the pair symmetry
cos(p,q)==cos(q,p) is folded into the per-pixel weights ww.

Per block n (y0 = 2+10n):
  g [C,4616] bf16 <- HBM window
  products: per (wave, slot) t = g * shift(g) on DVE (10 slots) or the
    square trick (slots 0,1: Pool add + ACT square, PE -n2 corrections)
    into [C, 5W] slot tiles from a 28-deep pool, so products pipeline a
    full block ahead of the PE dot matmuls.
  norms: 12 eye12 one-hot matmuls -> n2 psum [12,384]; Ln -> lnt bf16.
  dots: per wave 60 eye64 one-hot matmuls + 3 hw-correction matmuls
    accumulate 2*dot/dot into pd psum [128,384] (hw weights also write
    dead rows 60-63 so every psum row is finite).
  lp+lq: 6 eq one-hot matmuls accumulate ln|f_p| + ln|f_q| into one
    psum; ipq = Exp(-0.5 lpq) = 1/(|f_p||f_q|) in one ACT op.
  post: pk=copy(pd); z=pk*ipq; v=z-lab; w1=v*ww; w2=w1*v (Pool);
    ones-weight PE matmul accumulates sum_rows(w2) into a [1,384] psum
    across all blocks; host folds 1/cnt, 1/24, valid, 1/n_valid.
"""

import numpy as np
import ml_dtypes

import concourse.bass as bass
import concourse.mybir as mybir
from concourse.tile import TileContext
from concourse.vector_clock import ScopedClock
from concourse import bass_utils

BF16 = ml_dtypes.bfloat16
F32 = mybir.dt.float32
BF = mybir.dt.bfloat16

W = 384
C = 128
SHIFTS = [(0, 1), (0, 2),
          (1, -2), (1, -1), (1, 0), (1, 1), (1, 2),
          (2, -2), (2, -1), (2, 0), (2, 1), (2, 2)]
OFFS = [dy * W + dx for dy, dx in SHIFTS]
NSH = 12
RPB = 10
NROWS = 128
SQSLOTS = (0, 1)
GLEN = 4616
NBLK = 19
NPIX = 194 * W
XPAD = 16
TS_BUFS = 14


def _patch_tile_drain():
    if getattr(TileContext, "_drain_patched", False):
        return

    def _drain_and_barrier(self, tick_clock, wait_clock):
        drain_inst = self.nc.sync.drain()
        wait_clock.add_sem_waits(
            drain_inst.ins, ScopedClock({None: tick_clock.global_clock}))
        si = drain_inst.ins.sync_info
        if si is not None and si.on_wait and len(si.on_wait) > 1:
            waits = list(si.on_wait)
            drain_inst.ins.sync_info = mybir.SyncInfo(
                on_wait=[waits[-1]], on_update=list(si.on_update or []))
            for w in waits[:-1]:
                nop = self.nc.sync.nop(nofuse=True)
                nop.ins.sync_info = mybir.SyncInfo(on_wait=[w], on_update=[])
        self.nc.all_engine_barrier()
        popped = self.nc._tile_sem_poison_stack.pop()
        assert popped is self._sem_poison
        self.nc.clear_and_free_semaphores(list(self.sems.allocated().values()))
        self.nc.all_engine_barrier()

    TileContext._drain_and_barrier = _drain_and_barrier
    TileContext._drain_patched = True


_WSPLIT_N = [0]


def _split_multi_waits(nc, max_waits=1):
    """This container's walrus rejects instructions with more than one sync
    wait; hoist excess waits onto same-engine NOPs inserted just before."""
    for fn in nc.m.functions:
        for blk in fn.blocks:
            insts = blk.instructions
            out = []
            for inst in insts:
                si = inst.sync_info
                if si is not None and si.on_wait and len(si.on_wait) > max_waits:
                    waits = list(si.on_wait)
                    keep = waits[-max_waits:]
                    for w in waits[:-max_waits]:
                        _WSPLIT_N[0] += 1
                        nop = mybir.InstNoOp(
                            name=f"wsplit_{_WSPLIT_N[0]}", ins=[], outs=[])
                        nop.engine = inst.engine
                        nop.sync_info = mybir.SyncInfo(on_wait=[w],
                                                       on_update=[])
                        out.append(nop)
                    inst.sync_info = mybir.SyncInfo(
                        on_wait=keep, on_update=list(si.on_update or []))
                out.append(inst)
            blk.instructions = out


def build_nc(nblk=NBLK, repeat=1):
    _patch_tile_drain()
    nc = bass.Bass()
    x = nc.dram_tensor("x", [C, NPIX + XPAD], BF, kind="ExternalInput")
    labw = nc.dram_tensor("labw", [nblk, NROWS, 2 * W], BF,
                          kind="ExternalInput")
    eye12 = nc.dram_tensor("eye12", [C, NSH, NSH], BF, kind="ExternalInput")
    eye64 = nc.dram_tensor("eye64", [C, 64, 64], BF, kind="ExternalInput")
    eq = nc.dram_tensor("eq", [NSH, 6, NROWS], BF, kind="ExternalInput")
    hwt = nc.dram_tensor("hwt", [NSH, 6, 64], BF, kind="ExternalInput")
    onesw = nc.dram_tensor("onesw", [NROWS, 1], BF, kind="ExternalInput")
    out = nc.dram_tensor("out", [1, W], F32, kind="ExternalOutput")

    with TileContext(nc) as tc:
        with (tc.tile_pool(name="const", bufs=1) as cpool,
              tc.tile_pool(name="gbuf", bufs=2) as gpool,
              tc.tile_pool(name="sqp", bufs=2) as sqpool,
              tc.tile_pool(name="tp", bufs=TS_BUFS) as tpool,
              tc.tile_pool(name="apl", bufs=4) as apool,
              tc.tile_pool(name="inv", bufs=2) as ipool,
              tc.tile_pool(name="post", bufs=2) as postpool,
              tc.tile_pool(name="npsum", bufs=2, space="PSUM") as npsum,
              tc.tile_pool(name="dpsum", bufs=2, space="PSUM") as dpsum,
              tc.tile_pool(name="lpsum", bufs=2, space="PSUM") as lpsum,
              tc.tile_pool(name="wpsum", bufs=1, space="PSUM") as wpsum):

            eye12_sb = cpool.tile([C, NSH, NSH], BF)
            nc.sync.dma_start(eye12_sb[:], eye12[:])
            eye64_sb = cpool.tile([C, 64, 64], BF)
            nc.sync.dma_start(eye64_sb[:], eye64[:])
            eq_sb = cpool.tile([NSH, 6, NROWS], BF)
            nc.sync.dma_start(eq_sb[:], eq[:])
            hw_sb = cpool.tile([NSH, 6, 64], BF)
            nc.sync.dma_start(hw_sb[:], hwt[:])
            ones_sb = cpool.tile([NROWS, 1], BF)
            nc.sync.dma_start(ones_sb[:], onesw[:])
            wacc = wpsum.tile([1, W], F32)

            def load_g(n):
                win0 = (2 + RPB * n) * W
                g = gpool.tile([C, GLEN], BF, tag="g", name=f"g{n}")
                nc.sync.dma_start(g[:], x[:, win0:win0 + GLEN])
                return g

            def square(g, n):
                sq = sqpool.tile([C, 12 * W], BF, tag="sq", name=f"sq{n}")
                nc.scalar.square(sq[:], g[:, 0:12 * W])
                return sq

            def products(g, it):
                ts = [[None] * NSH, [None] * NSH]
                slot_order = list(range(2, NSH)) + list(SQSLOTS)
                for w in range(2):
                    base = w * 5 * W
                    for s in slot_order:
                        off = OFFS[s]
                        t = tpool.tile([C, 5 * W], BF, tag="ts",
                                       name=f"t{it}_{w}_{s}")
                        ts[w][s] = t
                        if s in SQSLOTS:
                            a = apool.tile([C, 5 * W], BF, tag="a",
                                           name=f"a{it}_{w}_{s}")
                            nc.gpsimd.tensor_add(
                                a[:], g[:, base:base + 5 * W],
                                g[:, base + off:base + off + 5 * W])
                            nc.scalar.square(t[:], a[:])
                        else:
                            nc.vector.tensor_mul(
                                t[:], g[:, base:base + 5 * W],
                                g[:, base + off:base + off + 5 * W])
                return ts

            g = load_g(0)
            sq = square(g, 0)
            ts = products(g, 0)
            prev = None
            pool_post = None

            total = nblk * repeat
            for it in range(total):
                n = it % nblk
                first = it <= 1

                if prev is not None:
                    pool_post = _emit_copies(nc, postpool, *prev)
                    prev = None

                # ---- norms ----
                n2 = npsum.tile([NSH, W], F32, tag="n2", name=f"n2_{it}")
                for j in range(12):
                    nc.tensor.matmul(n2[:], eye12_sb[:, j, :],
                                     sq[:, j * W:(j + 1) * W],
                                     start=(j == 0), stop=(j == 11))
                n2sb = ipool.tile([NSH, W + 4], BF, tag="n2sb",
                                  name=f"n2sb{it}")
                nc.scalar.copy(n2sb[:, 2:W + 2], n2[:])
                lnt = ipool.tile([NSH, W + 4], BF, tag="lnt",
                                 name=f"lnt{it}")
                nc.scalar.activation(lnt[:, 2:W + 2], n2[:],
                                     mybir.ActivationFunctionType.Ln)
                if first:
                    nc.vector.memset(lnt[:, 0:2], 1e4)
                    nc.vector.memset(lnt[:, W + 2:W + 4], 1e4)
                    nc.vector.memset(n2sb[:, 0:2], 0.0)
                    nc.vector.memset(n2sb[:, W + 2:W + 4], 0.0)

                # ---- dot matmuls ----
                pd = dpsum.tile([NROWS, W], F32, tag="pd", name=f"pd{it}")
                for w in range(2):
                    base_row = 64 * w
                    rows = list(range(10, 60)) + list(range(0, 10))
                    for i, row in enumerate(rows):
                        s, r5 = row // 5, row % 5
                        nc.tensor.matmul(
                            pd[base_row:base_row + 64, :],
                            eye64_sb[:, row, :],
                            ts[w][s][:, r5 * W:(r5 + 1) * W],
                            start=(i == 0), stop=False)
                    for i, (hj, dxo) in enumerate([(0, 0), (1, 1), (2, 2)]):
                        nc.tensor.matmul(
                            pd[base_row:base_row + 64, :],
                            hw_sb[:, 3 * w + hj, :],
                            n2sb[:, 2 + dxo:2 + dxo + W],
                            start=False, stop=(i == 2))

                # ---- lp + lq accumulation ----
                lpq = lpsum.tile([NROWS, W], F32, tag="lpq",
                                 name=f"lpq{it}")
                nc.tensor.matmul(lpq[:], eq_sb[:, 0, :],
                                 lnt[:, 2:W + 2], start=True, stop=False)
                for di in range(5):
                    nc.tensor.matmul(lpq[:], eq_sb[:, 1 + di, :],
                                     lnt[:, di:di + W],
                                     start=False, stop=(di == 4))

                # ---- prefetch next block ----
                if it + 1 < total:
                    gn = load_g((it + 1) % nblk)
                    sqn = square(gn, it + 1)
                    tsn = products(gn, it + 1)
                else:
                    gn = sqn = tsn = None

                # ---- chain for it-1 + its PE reduce ----
                if pool_post is not None:
                    w2t = _emit_pool_chain(nc, cpool, *pool_post)
                    nc.tensor.matmul(wacc[:], ones_sb[:], w2t[:],
                                     start=(it == 1), stop=False)
                    pool_post = None
                lw = postpool.tile([NROWS, 2 * W], BF, tag="lw",
                                   name=f"lw{it}")
                nc.sync.dma_start(lw[:], labw[n])
                prev = (pd, lpq, lw, n)
                g, sq, ts = gn, sqn, tsn

            pool_post = _emit_copies(nc, postpool, *prev)
            w2t = _emit_pool_chain(nc, cpool, *pool_post)
            nc.tensor.matmul(wacc[:], ones_sb[:], w2t[:],
                             start=False, stop=True)
            osb = cpool.tile([1, W], F32)
            nc.scalar.copy(osb[:], wacc[:])
            nc.sync.dma_start(out[:], osb[:])
    _split_multi_waits(nc)
    return nc


def _emit_copies(nc, postpool, pd, lpq, lw, n):
    pk = postpool.tile([NROWS, W], BF, tag="pk", name=f"pk{n}")
    nc.scalar.copy(pk[:], pd[:])
    ipq = postpool.tile([NROWS, W], BF, tag="ipq", name=f"ipq{n}")
    nc.scalar.activation(ipq[:], lpq[:],
                         mybir.ActivationFunctionType.Exp, scale=-0.5)
    return (pk, ipq, lw, n)


def _emit_pool_chain(nc, cpool, pk, ipq, lw, n):
    z = cpool.tile([NROWS, W], BF, tag="z", bufs=2, name=f"z{n}")
    nc.gpsimd.tensor_mul(z[:], pk[:], ipq[:])
    v = cpool.tile([NROWS, W], BF, tag="v", bufs=2, name=f"v{n}")
    nc.gpsimd.tensor_sub(v[:], z[:], lw[:, 0:W])
    w1 = cpool.tile([NROWS, W], BF, tag="w1", bufs=2, name=f"w1_{n}")
    nc.gpsimd.tensor_mul(w1[:], v[:], lw[:, W:2 * W])
    w2 = cpool.tile([NROWS, W], BF, tag="w2", bufs=2, name=f"w2_{n}")
    nc.gpsimd.tensor_mul(w2[:], w1[:], v[:])
    return w2


def make_consts():
    eye12 = np.broadcast_to(np.eye(NSH, dtype=BF16), (C, NSH, NSH)).copy()
    eye64 = np.broadcast_to(np.eye(64, dtype=BF16), (C, 64, 64)).copy()
    eq = np.zeros((6, NSH, NROWS), BF16)
    for w in range(2):
        for s in range(NSH):
            dy, dx = SHIFTS[s]
            for r5 in range(5):
                row = 64 * w + 5 * s + r5
                j = 5 * w + r5
                eq[0, j, row] = 1                    # lp: ln n2 at produced
                eq[1 + (dx + 2), j + dy, row] = 1    # lq: ln n2 at partner
    # hwm[w*3 + k]: -1 one-hots for the square-trick norm corrections;
    # group row 0 also writes +n2 into dead psum rows 60-63 (finite; the
    # zero lab/ww rows null them out of the final reduce).
    hwm = np.zeros((6, NSH, 64), BF16)
    for w in range(2):
        for si, s in enumerate(SQSLOTS):
            dy, dx = SHIFTS[s]
            for r5 in range(5):
                col = 5 * s + r5
                j = 5 * w + r5
                hwm[3 * w + 0, j, col] = -1            # -n2 at produced
                hwm[3 * w + 1 + si, j + dy, col] = -1  # -n2 at partner
        hwm[3 * w + 0, 0, 60:64] = 1
    onesw = np.ones((NROWS, 1), BF16)
    return (eye12, eye64, np.ascontiguousarray(eq.transpose(1, 0, 2)),
            np.ascontiguousarray(hwm.transpose(1, 0, 2)), onesw)


def host_prep(er_input, seg_label, gt_boundary_seg, nblk=NBLK):
    B, _, H, Wd = er_input.shape
    f32 = np.float32
    gb = np.where(gt_boundary_seg == 255, 0, gt_boundary_seg)
    slc = np.where(seg_label == 255, 0, seg_label)
    gt_b1 = gb * slc[:, 1]
    boundary = gt_b1 > 0
    iy = np.arange(H)
    ix = np.arange(Wd)
    interior = (((iy >= 2) & (iy <= H - 3))[:, None]
                & ((ix >= 2) & (ix <= Wd - 3))[None, :])
    sel = boundary & interior
    cnt = sel.sum(axis=(1, 2)).astype(f32)
    valid = boundary.sum(axis=(1, 2)) >= 1
    n_valid = valid.astype(f32).sum()

    seg_f = seg_label.astype(f32)
    lab_stack = np.empty((NSH, B, H, Wd), f32)
    w_stack = np.empty((NSH, B, H, Wd), f32)
    sel_f = sel.astype(f32)
    for m, (dy, dx) in enumerate(SHIFTS):
        rolled = np.roll(seg_f, (-dy, -dx), axis=(2, 3))
        lab_stack[m] = (seg_f * rolled).sum(axis=1)
        sh = np.zeros_like(sel_f)
        ys0, ys1 = max(0, -dy), min(H, H - dy)
        xs0, xs1 = max(0, -dx), min(Wd, Wd - dx)
        sh[:, ys0:ys1, xs0:xs1] = sel_f[:, ys0 + dy:ys1 + dy,
                                        xs0 + dx:xs1 + dx]
        w_stack[m] = sel_f + sh
    # square-trick slots: kernel computes 2*dot -> z = 2*cos
    for s in SQSLOTS:
        lab_stack[s] *= 2.0
        w_stack[s] *= 0.25

    eye12, eye64, eqm, hwm, onesw = make_consts()
    per_core = []
    for k in range(8):
        b, h = k // 2, k % 2
        r0 = 0 if h == 0 else 190
        xs = np.zeros((C, NPIX + XPAD), BF16)
        xs[:, :NPIX] = er_input[b, :, r0:r0 + 194, :].reshape(C, -1)
        gr = (r0 + 2 + RPB * np.arange(nblk)[:, None, None]
              + 5 * np.arange(2)[None, :, None]
              + np.arange(5)[None, None, :])          # [nblk, 2, 5]
        lab_g = lab_stack[:, b][:, gr]                # [NSH, nblk, 2, 5, W]
        ww_g = w_stack[:, b][:, gr]
        lwc = np.zeros((nblk, NROWS, 2 * Wd), BF16)
        rowidx = (64 * np.arange(2)[:, None, None]
                  + 5 * np.arange(NSH)[None, :, None]
                  + np.arange(5)[None, None, :]).reshape(-1)
        lab_p = lab_g.transpose(1, 2, 0, 3, 4).reshape(nblk, 120, Wd)
        ww_p = ww_g.transpose(1, 2, 0, 3, 4).reshape(nblk, 120, Wd)
        lwc[:, rowidx, 0:Wd] = lab_p
        lwc[:, rowidx, Wd:2 * Wd] = ww_p
        per_core.append({"x": xs, "labw": lwc, "eye12": eye12,
                         "eye64": eye64, "eq": eqm, "hwt": hwm,
                         "onesw": onesw})
    return per_core, dict(cnt=cnt, valid=valid, n_valid=n_valid)


def finish(core_sums, meta):
    f32 = np.float32
    cnt, valid, n_valid = meta["cnt"], meta["valid"], meta["n_valid"]
    total = f32(0.0)
    for b in range(4):
        sb = f32(core_sums[2 * b] + core_sums[2 * b + 1])
        loss_b = sb / max(cnt[b], f32(1.0)) / f32(24.0)
        if valid[b]:
            total = total + loss_b
    total = total / max(n_valid, f32(1.0))
    if np.isnan(total):
        total = f32(0.0)
    return np.float32(total)


_NC_CACHE = {}


def kernel(er_input, seg_label, gt_boundary_seg):
    er_input = np.asarray(er_input)
    seg_label = np.asarray(seg_label)
    gt_boundary_seg = np.asarray(gt_boundary_seg)
    per_core, meta = host_prep(er_input, seg_label, gt_boundary_seg)
    if "nc" not in _NC_CACHE:
        _NC_CACHE["nc"] = build_nc()
    nc = _NC_CACHE["nc"]
    res = bass_utils.run_bass_kernel_spmd(nc, per_core,
                                          core_ids=list(range(8)))
    sums = [r["out"].astype(np.float64).sum() for r in res.results]
    return finish(sums, meta)


# revision 3
# speedup vs baseline: 1.9684x; 1.9684x over previous
"""Bass TRN2 kernel for the boundary cosine-similarity context loss.

All-DVE-products variant: 12 shift-products per wave as direct DVE
tensor_mul ops into [C,5W] slot tiles (24 ops/block, pool bufs=14);
norms via 12 one-hot matmuls + Ln -> lnt bf16; dots via 120 one-hot
matmuls + d4 dead-row matmul per wave; fused lp+lq psum + single
Exp -> ipq; Pool post chain; ones-weight PE matmul accumulates the
weighted sum into a persistent [1,384] psum. Host folds 1/cnt, 1/24,
valid, 1/n_valid. (No square trick: GpSimd shares an SBUF port with
VectorE, so Pool product adds serialized against DVE products.)
"""

import numpy as np
import ml_dtypes

import concourse.bass as bass
import concourse.mybir as mybir
from concourse.tile import TileContext
from concourse.vector_clock import ScopedClock
from concourse import bass_utils

BF16 = ml_dtypes.bfloat16
F32 = mybir.dt.float32
BF = mybir.dt.bfloat16

W = 384
C = 128
SHIFTS = [(0, 1), (0, 2),
          (1, -2), (1, -1), (1, 0), (1, 1), (1, 2),
          (2, -2), (2, -1), (2, 0), (2, 1), (2, 2)]
OFFS = [dy * W + dx for dy, dx in SHIFTS]
NSH = 12
RPB = 10
NROWS = 128
GLEN = 4616
NBLK = 19
NPIX = 194 * W
XPAD = 16
TS_BUFS = 14

def _patch_tile_drain():
    if getattr(TileContext, "_drain_patched", False):
        return

    def _drain_and_barrier(self, tick_clock, wait_clock):
        drain_inst = self.nc.sync.drain()
        wait_clock.add_sem_waits(
            drain_inst.ins, ScopedClock({None: tick_clock.global_clock}))
        si = drain_inst.ins.sync_info
        if si is not None and si.on_wait and len(si.on_wait) > 1:
            waits = list(si.on_wait)
            drain_inst.ins.sync_info = mybir.SyncInfo(
                on_wait=[waits[-1]], on_update=list(si.on_update or []))
            for w in waits[:-1]:
                nop = self.nc.sync.nop(nofuse=True)
                nop.ins.sync_info = mybir.SyncInfo(on_wait=[w], on_update=[])
        self.nc.all_engine_barrier()
        popped = self.nc._tile_sem_poison_stack.pop()
        assert popped is self._sem_poison
        self.nc.clear_and_free_semaphores(list(self.sems.allocated().values()))
        self.nc.all_engine_barrier()

    TileContext._drain_and_barrier = _drain_and_barrier
    TileContext._drain_patched = True


_WSPLIT_N = [0]


def _split_multi_waits(nc, max_waits=1):
    """This container's walrus rejects instructions with more than one sync
    wait; hoist excess waits onto same-engine NOPs inserted just before."""
    for fn in nc.m.functions:
        for blk in fn.blocks:
            insts = blk.instructions
            out = []
            for inst in insts:
                si = inst.sync_info
                if si is not None and si.on_wait and len(si.on_wait) > max_waits:
                    waits = list(si.on_wait)
                    keep = waits[-max_waits:]
                    for w in waits[:-max_waits]:
                        _WSPLIT_N[0] += 1
                        nop = mybir.InstNoOp(
                            name=f"wsplit_{_WSPLIT_N[0]}", ins=[], outs=[])
                        nop.engine = inst.engine
                        nop.sync_info = mybir.SyncInfo(on_wait=[w],
                                                       on_update=[])
                        out.append(nop)
                    inst.sync_info = mybir.SyncInfo(
                        on_wait=keep, on_update=list(si.on_update or []))
                out.append(inst)
            blk.instructions = out


def make_consts():
    eye12 = np.broadcast_to(np.eye(NSH, dtype=BF16), (C, NSH, NSH)).copy()
    eye64 = np.broadcast_to(np.eye(64, dtype=BF16), (C, 64, 64)).copy()
    eq = np.zeros((6, NSH, NROWS), BF16)
    for w in range(2):
        for s in range(NSH):
            dy, dx = SHIFTS[s]
            for r5 in range(5):
                row = 64 * w + 5 * s + r5
                j = 5 * w + r5
                eq[0, j, row] = 1
                eq[1 + (dx + 2), j + dy, row] = 1
    onesw = np.ones((NROWS, 1), BF16)
    d4 = np.zeros((C, 64), BF16)
    for j in range(60, 64):
        d4[j, j] = 1
    return (eye12, eye64, np.ascontiguousarray(eq.transpose(1, 0, 2)),
            onesw, d4)


def host_prep(er_input, seg_label, gt_boundary_seg, nblk=NBLK):
    B, _, H, Wd = er_input.shape
    f32 = np.float32
    gb = np.where(gt_boundary_seg == 255, 0, gt_boundary_seg)
    slc = np.where(seg_label == 255, 0, seg_label)
    gt_b1 = gb * slc[:, 1]
    boundary = gt_b1 > 0
    iy = np.arange(H)
    ix = np.arange(Wd)
    interior = (((iy >= 2) & (iy <= H - 3))[:, None]
                & ((ix >= 2) & (ix <= Wd - 3))[None, :])
    sel = boundary & interior
    cnt = sel.sum(axis=(1, 2)).astype(f32)
    valid = boundary.sum(axis=(1, 2)) >= 1
    n_valid = valid.astype(f32).sum()

    seg_f = seg_label.astype(f32)
    lab_stack = np.empty((NSH, B, H, Wd), f32)
    w_stack = np.empty((NSH, B, H, Wd), f32)
    sel_f = sel.astype(f32)
    for m, (dy, dx) in enumerate(SHIFTS):
        rolled = np.roll(seg_f, (-dy, -dx), axis=(2, 3))
        lab_stack[m] = (seg_f * rolled).sum(axis=1)
        sh = np.zeros_like(sel_f)
        ys0, ys1 = max(0, -dy), min(H, H - dy)
        xs0, xs1 = max(0, -dx), min(Wd, Wd - dx)
        sh[:, ys0:ys1, xs0:xs1] = sel_f[:, ys0 + dy:ys1 + dy,
                                        xs0 + dx:xs1 + dx]
        w_stack[m] = sel_f + sh
    # no square-trick folding: all slots are direct dots -> z = cos

    eye12, eye64, eqm, onesw, d4 = make_consts()
    per_core = []
    for k in range(8):
        b, h = k // 2, k % 2
        r0 = 0 if h == 0 else 190
        xs = np.zeros((C, NPIX + XPAD), BF16)
        xs[:, :NPIX] = er_input[b, :, r0:r0 + 194, :].reshape(C, -1)
        gr = (r0 + 2 + RPB * np.arange(nblk)[:, None, None]
              + 5 * np.arange(2)[None, :, None]
              + np.arange(5)[None, None, :])
        lab_g = lab_stack[:, b][:, gr]
        ww_g = w_stack[:, b][:, gr]
        lwc = np.zeros((nblk, NROWS, 2 * Wd), BF16)
        rowidx = (64 * np.arange(2)[:, None, None]
                  + 5 * np.arange(NSH)[None, :, None]
                  + np.arange(5)[None, None, :]).reshape(-1)
        lab_p = lab_g.transpose(1, 2, 0, 3, 4).reshape(nblk, 120, Wd)
        ww_p = ww_g.transpose(1, 2, 0, 3, 4).reshape(nblk, 120, Wd)
        lwc[:, rowidx, 0:Wd] = lab_p
        lwc[:, rowidx, Wd:2 * Wd] = ww_p
        per_core.append({"x": xs, "labw": lwc, "eye12": eye12,
                         "eye64": eye64, "eq": eqm, "onesw": onesw,
                         "d4": d4})
    return per_core, dict(cnt=cnt, valid=valid, n_valid=n_valid)


def build_nc(nblk=NBLK, repeat=1):
    _patch_tile_drain()
    nc = bass.Bass()
    x = nc.dram_tensor("x", [C, NPIX + XPAD], BF, kind="ExternalInput")
    labw = nc.dram_tensor("labw", [nblk, NROWS, 2 * W], BF,
                          kind="ExternalInput")
    eye12 = nc.dram_tensor("eye12", [C, NSH, NSH], BF, kind="ExternalInput")
    eye64 = nc.dram_tensor("eye64", [C, 64, 64], BF, kind="ExternalInput")
    eq = nc.dram_tensor("eq", [NSH, 6, NROWS], BF, kind="ExternalInput")
    d4 = nc.dram_tensor("d4", [C, 64], BF, kind="ExternalInput")
    onesw = nc.dram_tensor("onesw", [NROWS, 1], BF, kind="ExternalInput")
    out = nc.dram_tensor("out", [1, W], F32, kind="ExternalOutput")

    with TileContext(nc) as tc:
        with (tc.tile_pool(name="const", bufs=1) as cpool,
              tc.tile_pool(name="gbuf", bufs=2) as gpool,
              tc.tile_pool(name="sqp", bufs=2) as sqpool,
              tc.tile_pool(name="tp", bufs=TS_BUFS) as tpool,
              tc.tile_pool(name="inv", bufs=2) as ipool,
              tc.tile_pool(name="post", bufs=2) as postpool,
              tc.tile_pool(name="npsum", bufs=2, space="PSUM") as npsum,
              tc.tile_pool(name="dpsum", bufs=2, space="PSUM") as dpsum,
              tc.tile_pool(name="lpsum", bufs=2, space="PSUM") as lpsum,
              tc.tile_pool(name="wpsum", bufs=1, space="PSUM") as wpsum):

            eye12_sb = cpool.tile([C, NSH, NSH], BF)
            nc.sync.dma_start(eye12_sb[:], eye12[:])
            eye64_sb = cpool.tile([C, 64, 64], BF)
            nc.sync.dma_start(eye64_sb[:], eye64[:])
            eq_sb = cpool.tile([NSH, 6, NROWS], BF)
            nc.sync.dma_start(eq_sb[:], eq[:])
            d4_sb = cpool.tile([C, 64], BF)
            nc.sync.dma_start(d4_sb[:], d4[:])
            ones_sb = cpool.tile([NROWS, 1], BF)
            nc.sync.dma_start(ones_sb[:], onesw[:])
            wacc = wpsum.tile([1, W], F32)

            def load_g(n):
                win0 = (2 + RPB * n) * W
                g = gpool.tile([C, GLEN], BF, tag="g", name=f"g{n}")
                nc.sync.dma_start(g[:], x[:, win0:win0 + GLEN])
                return g

            def square(g, n):
                sq = sqpool.tile([C, 12 * W], BF, tag="sq", name=f"sq{n}")
                nc.scalar.square(sq[:], g[:, 0:12 * W])
                return sq

            def products(g, it):
                ts = [[None] * NSH, [None] * NSH]
                slot_order = list(range(NSH))
                for w in range(2):
                    base = w * 5 * W
                    for s in slot_order:
                        off = OFFS[s]
                        t = tpool.tile([C, 5 * W], BF, tag="ts",
                                       name=f"t{it}_{w}_{s}")
                        ts[w][s] = t
                        nc.vector.tensor_mul(
                            t[:], g[:, base:base + 5 * W],
                            g[:, base + off:base + off + 5 * W])
                return ts

            g = load_g(0)
            sq = square(g, 0)
            ts = products(g, 0)
            prev = None
            pool_post = None

            total = nblk * repeat
            for it in range(total):
                n = it % nblk
                first = it <= 1

                if prev is not None:
                    pool_post = _emit_copies(nc, postpool, *prev)
                    prev = None

                # ---- norms ----
                n2 = npsum.tile([NSH, W], F32, tag="n2", name=f"n2_{it}")
                for j in range(12):
                    nc.tensor.matmul(n2[:], eye12_sb[:, j, :],
                                     sq[:, j * W:(j + 1) * W],
                                     start=(j == 0), stop=(j == 11))
                lnt = ipool.tile([NSH, W + 4], BF, tag="lnt",
                                 name=f"lnt{it}")
                nc.scalar.activation(lnt[:, 2:W + 2], n2[:],
                                     mybir.ActivationFunctionType.Ln)
                if first:
                    nc.vector.memset(lnt[:, 0:2], 1e4)
                    nc.vector.memset(lnt[:, W + 2:W + 4], 1e4)

                # ---- dot matmuls (consumption matches allocation order) ----
                pd = dpsum.tile([NROWS, W], F32, tag="pd", name=f"pd{it}")
                for w in range(2):
                    base_row = 64 * w
                    for i in range(60):
                        s, r5 = i // 5, i % 5
                        nc.tensor.matmul(
                            pd[base_row:base_row + 64, :],
                            eye64_sb[:, 5 * s + r5, :],
                            ts[w][s][:, r5 * W:(r5 + 1) * W],
                            start=(i == 0), stop=False)
                    nc.tensor.matmul(pd[base_row:base_row + 64, :],
                                     d4_sb[:], g[:, 0:W],
                                     start=False, stop=True)

                # ---- lp + lq accumulation ----
                lpq = lpsum.tile([NROWS, W], F32, tag="lpq",
                                 name=f"lpq{it}")
                nc.tensor.matmul(lpq[:], eq_sb[:, 0, :],
                                 lnt[:, 2:W + 2], start=True, stop=False)
                for di in range(5):
                    nc.tensor.matmul(lpq[:], eq_sb[:, 1 + di, :],
                                     lnt[:, di:di + W],
                                     start=False, stop=(di == 4))

                # ---- prefetch next block ----
                if it + 1 < total:
                    gn = load_g((it + 1) % nblk)
                    sqn = square(gn, it + 1)
                    tsn = products(gn, it + 1)
                else:
                    gn = sqn = tsn = None

                # ---- chain for it-1 + its PE reduce ----
                if pool_post is not None:
                    w2t = _emit_pool_chain(nc, cpool, *pool_post)
                    nc.tensor.matmul(wacc[:], ones_sb[:], w2t[:],
                                     start=(it == 1), stop=False)
                    pool_post = None
                lw = postpool.tile([NROWS, 2 * W], BF, tag="lw",
                                   name=f"lw{it}")
                nc.sync.dma_start(lw[:], labw[n])
                prev = (pd, lpq, lw, n)
                g, sq, ts = gn, sqn, tsn

            pool_post = _emit_copies(nc, postpool, *prev)
            w2t = _emit_pool_chain(nc, cpool, *pool_post)
            nc.tensor.matmul(wacc[:], ones_sb[:], w2t[:],
                             start=False, stop=True)
            osb = cpool.tile([1, W], F32)
            nc.scalar.copy(osb[:], wacc[:])
            nc.sync.dma_start(out[:], osb[:])
    _split_multi_waits(nc)
    return nc


def _emit_copies(nc, postpool, pd, lpq, lw, n):
    pk = postpool.tile([NROWS, W], BF, tag="pk", name=f"pk{n}")
    nc.scalar.copy(pk[:], pd[:])
    ipq = postpool.tile([NROWS, W], BF, tag="ipq", name=f"ipq{n}")
    nc.scalar.activation(ipq[:], lpq[:],
                         mybir.ActivationFunctionType.Exp, scale=-0.5)
    return (pk, ipq, lw, n)


def _emit_pool_chain(nc, cpool, pk, ipq, lw, n):
    z = cpool.tile([NROWS, W], BF, tag="z", bufs=2, name=f"z{n}")
    nc.gpsimd.tensor_mul(z[:], pk[:], ipq[:])
    v = cpool.tile([NROWS, W], BF, tag="v", bufs=2, name=f"v{n}")
    nc.gpsimd.tensor_sub(v[:], z[:], lw[:, 0:W])
    w1 = cpool.tile([NROWS, W], BF, tag="w1", bufs=2, name=f"w1_{n}")
    nc.gpsimd.tensor_mul(w1[:], v[:], lw[:, W:2 * W])
    w2 = cpool.tile([NROWS, W], BF, tag="w2", bufs=2, name=f"w2_{n}")
    nc.gpsimd.tensor_mul(w2[:], w1[:], v[:])
    return w2


def finish(core_sums, meta):
    f32 = np.float32
    cnt, valid, n_valid = meta["cnt"], meta["valid"], meta["n_valid"]
    total = f32(0.0)
    for b in range(4):
        sb = f32(core_sums[2 * b] + core_sums[2 * b + 1])
        loss_b = sb / max(cnt[b], f32(1.0)) / f32(24.0)
        if valid[b]:
            total = total + loss_b
    total = total / max(n_valid, f32(1.0))
    if np.isnan(total):
        total = f32(0.0)
    return np.float32(total)


_NC_CACHE = {}


def kernel(er_input, seg_label, gt_boundary_seg):
    er_input = np.asarray(er_input)
    seg_label = np.asarray(seg_label)
    gt_boundary_seg = np.asarray(gt_boundary_seg)
    per_core, meta = host_prep(er_input, seg_label, gt_boundary_seg)
    if "nc" not in _NC_CACHE:
        _NC_CACHE["nc"] = build_nc()
    nc = _NC_CACHE["nc"]
    res = bass_utils.run_bass_kernel_spmd(nc, per_core,
                                          core_ids=list(range(8)))
    sums = [r["out"].astype(np.float64).sum() for r in res.results]
    return finish(sums, meta)
